# revision 40
# baseline (speedup 1.0000x reference)
"""BiAttention (BiDAF-style) kernel for Trainium2, 8 NeuronCores.

Reference math (T=4096, d=512):
    context  = x[0,0]; question = x[1,0]
    S[i,j]   = w1.c_i + w2.q_j + (c_i*w3).q_j
    A        = softmax_j(S)          # w1.c_i is constant per row -> cancels
    U_A      = A @ question
    b        = max_j A[i,j]
    h        = b @ context           # global over T -> one AllReduce
    G        = [context, U_A, context*U_A, context*h]

Sharding: context rows (rows of S/A/U_A/G) split across 8 cores (512 each);
question replicated; h all-reduced (2 KB).

Per-core compute strategy (all big GEMMs in fp8-e4m3 DoubleRow, 4x bf16
PE throughput; S computed TRANSPOSED so exp emits E^T directly and no
E transposes are needed):

  S^T[j,i] = sum_d q[j,d] * (c[i,d]*w3[d] + w2[d])
    - lhsT  = qT (d on partitions), host-pretransposed fp8 hi + lo parts
      (q = q8 + qlo8 error-compensation: halves the fp8 matmul noise,
      needed for the b/h accuracy budget)
    - rhs   = cw3T fp8 = (c^T * w3 + w2), from 16 PE transposes of bf16 c
    - 4 DoubleRow matmuls per 128-row j-tile (2 d-pairs x {hi,lo})
  E^T = exp(S^T - 2) -> bf16 SBUF (ACT; global shift keeps E in fp8 range,
    softmax/max ratios are shift-invariant)
  E8  = fp8(E^T)     (Pool/DVE copies; feeds the U_A GEMM)
  Z   = e8-chunk^T @ ones per i-block: tiny DoubleRow column matmuls give
        Z directly in the [p, ib] layout, no cross-partition reduce
  b   = max_j E^T / Z: DVE running tensor_max per tile, then 4 PE
        transposes of the [j-lane, i] partial max + one free-axis DVE max
        (bf16-accurate; an fp8-E max would blow the tolerance)
  U_A^T[dq,i] = sum_j q8[j,dq] * E8[j,i]  (lhsT = q natural fp8 - no
        transposes; 64 DoubleRow matmuls), then 16 PE transposes back
        and scale by 1/Z.
  h   = b @ c  (tiny bf16 matmuls, f32 psum, 2 KB in-place AllReduce)
  G0  = c exactly via DRAM->DRAM copy; G1..3 written bf16 (within the
        2e-2 budget) and upcast on host.

The schedule keeps every engine off the b->h->G3 critical path: the h
round-trip DMAs ride the otherwise-idle sync queue, output writes ride
the scalar queue, and the U_A pair matmuls trail their e8 casts by two
tiles so the in-order PE queue never head-of-line blocks.
"""

import numpy as np
import ml_dtypes

import concourse.bass as bass
import concourse.mybir as mybir
import concourse.tile as tile
from concourse import bacc
from concourse.bass_utils import run_bass_kernel_spmd
from concourse.masks import make_identity

F32 = mybir.dt.float32
BF16 = mybir.dt.bfloat16
F8 = mybir.dt.float8e4
AF = mybir.ActivationFunctionType

T = 4096
D = 512
NCORES = 8
TL = T // NCORES          # 512 local context rows per core
P = 128
NIB = TL // P             # 4 i-blocks of 128 rows
NJT = T // P              # 32 j-tiles of 128
NDC = D // P              # 4 d-chunks of 128
SHIFT = 2.0               # global logit shift: E = exp(S - 2) <= ~13


def build_kernel(collective=True, compile=True):
    nc = bacc.Bacc("TRN2", target_bir_lowering=False, debug=False,
                   num_devices=NCORES if collective else 1)

    c_dram = nc.dram_tensor("c", [TL, D], F32, kind="ExternalInput").ap()
    cb_dram = nc.dram_tensor("cb", [P, NIB, D], BF16, kind="ExternalInput").ap()
    q8n_dram = nc.dram_tensor("q8n", [P, NJT, D], F8, kind="ExternalInput").ap()
    qth_dram = nc.dram_tensor("qth", [P, NDC, T], F8, kind="ExternalInput").ap()
    qtl_dram = nc.dram_tensor("qtl", [P, NDC, T], F8, kind="ExternalInput").ap()
    w2p_dram = nc.dram_tensor("w2p", [P, NDC], F32, kind="ExternalInput").ap()
    w3p_dram = nc.dram_tensor("w3p", [P, NDC], F32, kind="ExternalInput").ap()
    g0_dram = nc.dram_tensor("g0", [TL, D], F32, kind="ExternalOutput").ap()
    g123_dram = nc.dram_tensor("g123", [TL, 3 * D], BF16,
                               kind="ExternalOutput").ap()

    with tile.TileContext(nc) as tc:
        _emit(nc, tc, c_dram, cb_dram, q8n_dram, qth_dram, qtl_dram, w2p_dram,
              w3p_dram, g0_dram, g123_dram, collective=collective)

    if compile:
        nc.compile()
    return nc


def _emit(nc, tc, c_dram, cb_dram, q8n_dram, qth_dram, qtl_dram, w2p_dram,
          w3p_dram, g0_dram, g123_dram, collective=True):
    from contextlib import ExitStack
    ctx = ExitStack()
    consts = ctx.enter_context(tc.tile_pool(name="consts", bufs=1))
    epool = ctx.enter_context(tc.tile_pool(name="epool", bufs=1))
    gout = ctx.enter_context(tc.tile_pool(name="gout", bufs=2))
    stat = ctx.enter_context(tc.tile_pool(name="stat", bufs=2))
    spool = ctx.enter_context(tc.tile_pool(name="spool", bufs=4, space="PSUM"))
    uapool = ctx.enter_context(tc.tile_pool(name="uapool", bufs=1, space="PSUM"))
    dram = ctx.enter_context(tc.tile_pool(name="dram", bufs=1, space="DRAM"))

    # ---- prologue --------------------------------------------------------
    # cb load: [p, ib, d] bf16, host-precast; gates cw3T which gates all S^T
    cb = consts.tile([P, NIB, D], BF16)
    nc.sync.dma_start(out=cb[:, :, 0:D // 2], in_=cb_dram[:, :, 0:D // 2])
    nc.sync.dma_start(out=cb[:, :, D // 2:D], in_=cb_dram[:, :, D // 2:D])

    ident = consts.tile([P, P], BF16)
    make_identity(nc, ident)

    # dummy exp: pull the ACT table load into the startup DMA window
    warm = consts.tile([1, 1], F32)
    nc.vector.memset(warm, 0.0)
    nc.scalar.activation(out=warm, in_=warm, func=AF.Exp)

    ebias = consts.tile([P, 1], F32)
    nc.vector.memset(ebias, -SHIFT)
    ones8 = consts.tile([P, 2, 1], F8)
    nc.vector.memset(ones8, 1.0)

    # HAM warm-up: ramp the PE clock while startup DMAs run
    wa = consts.tile([P, P], BF16)
    nc.vector.memset(wa, 0.0)
    wb = consts.tile([P, 512], BF16)
    nc.vector.memset(wb, 0.0)
    for wi in range(3):
        wps = uapool.tile([P, 512], F32, tag=f"ua{wi}", name=f"wps{wi}")
        nc.tensor.matmul(wps, lhsT=wa, rhs=wb, start=True, stop=True)

    # w first (tiny; gates the cw3T ACT drain), then 8 interleaved groups of
    # (qth, qtl) 4-tile slices with q8n trailing one group (UA lags anyway),
    # so the S^T pipeline starts after ~4 small DMAs instead of all of q
    w2p = consts.tile([P, NDC], F32)
    nc.sync.dma_start(out=w2p, in_=w2p_dram)
    w3p = consts.tile([P, NDC], F32)
    nc.sync.dma_start(out=w3p, in_=w3p_dram)
    qth = consts.tile([P, NDC, T], F8)
    qtl = consts.tile([P, NDC, T], F8)
    q8n = consts.tile([P, NJT, D], F8)
    nc.sync.dma_start(out=qth[:, :, 0:512], in_=qth_dram[:, :, 0:512])
    nc.sync.dma_start(out=qtl[:, :, 0:512], in_=qtl_dram[:, :, 0:512])
    for g in range(8):
        j0, j1 = g * 512, (g + 1) * 512
        if g >= 1:
            nc.sync.dma_start(out=qth[:, :, j0:j1], in_=qth_dram[:, :, j0:j1])
            nc.sync.dma_start(out=qtl[:, :, j0:j1], in_=qtl_dram[:, :, j0:j1])
        if g >= 1:
            nc.sync.dma_start(out=q8n[:, (g - 1) * 4:g * 4, :],
                              in_=q8n_dram[:, (g - 1) * 4:g * 4, :])
    nc.sync.dma_start(out=q8n[:, 28:32, :], in_=q8n_dram[:, 28:32, :])
    # G0 = context, exact f32, DRAM->DRAM; sync queue AFTER the q loads so
    # its (dependency-free) transfer cannot preempt the pipeline feed
    nc.sync.dma_start(out=g0_dram, in_=c_dram)

    # ---- cw3T8[d, i] = c^T * w3 + w2, fp8 --------------------------------
    cw3 = consts.tile([P, NDC, TL], F8)   # [p(d), dc, i]
    for dc in range(NDC):
        ps = uapool.tile([P, TL], BF16, tag=f"ua{dc}", name=f"tc{dc}")
        for ib in range(NIB):
            nc.tensor.transpose(ps[:, ib * P:(ib + 1) * P],
                                cb[:, ib, dc * P:(dc + 1) * P], ident)
        nc.scalar.activation(out=cw3[:, dc, :], in_=ps, func=AF.Identity,
                             bias=w2p[:, dc:dc + 1], scale=w3p[:, dc:dc + 1])

    # ---- persistent tiles ------------------------------------------------
    e16 = epool.tile([P, NJT, TL], BF16, tag="e16", name="e16")  # [j, jt, i]
    e8 = epool.tile([P, NJT, TL], F8, tag="e8", name="e8")
    macc = stat.tile([P, TL], BF16, tag="macc", name="macc")
    ua_ps = [uapool.tile([P, TL], F32, tag=f"ua{dqc}", name=f"ua{dqc}")
             for dqc in range(NDC)]

    # ---- main loop over j-tile pairs -------------------------------------
    def emit_pair(tp):
        """U_A DoubleRow matmuls for j-tile pair tp (tiles 2tp, 2tp+1)."""
        jt0 = 2 * tp
        for dqc in range(NDC):
            nc.tensor.matmul(
                ua_ps[dqc],
                lhsT=q8n[:, jt0:jt0 + 2, dqc * P:(dqc + 1) * P],
                rhs=e8[:, jt0:jt0 + 2, :],
                start=(tp == 0), stop=(tp == NJT // 2 - 1),
                perf_mode=mybir.MatmulPerfMode.DoubleRow,
                skip_group_check=True)

    for jt in range(NJT):
        st = spool.tile([P, TL], F32, tag="st2", name=f"st{jt}")
        m = 0
        for qt in (qth, qtl):
            for cp in range(2):
                nc.tensor.matmul(
                    st,
                    lhsT=qt[:, 2 * cp:2 * cp + 2, jt * P:(jt + 1) * P],
                    rhs=cw3[:, 2 * cp:2 * cp + 2, :],
                    start=(m == 0), stop=(m == 3),
                    perf_mode=mybir.MatmulPerfMode.DoubleRow)
                m += 1
        nc.scalar.activation(out=e16[:, jt, :], in_=st, func=AF.Exp,
                             bias=ebias, scale=1.0)
        if jt == 0:
            nc.vector.tensor_copy(out=macc, in_=e16[:, jt, :])
        else:
            nc.vector.tensor_max(out=macc, in0=macc, in1=e16[:, jt, :])
        cast_eng = nc.gpsimd if jt % 8 in (0, 1, 3, 4, 6) else nc.vector
        cast_eng.tensor_copy(out=e8[:, jt, :], in_=e16[:, jt, :])
        # UA pairs run behind the casts so the in-order PE queue never
        # head-of-line blocks on a fresh e8 cast; pairs 14/15 land after
        # the b/h launch to fill the AllReduce latency window with U_A work
        if jt % 2 == 1 and 5 <= jt <= 27:
            emit_pair((jt - 5) // 2)

    # ---- stats: maxe via PE transpose of the running max (partition max
    # without gpsimd: transpose [j-lane, i] -> [i, j-lane], then a free-axis
    # DVE max directly in the [p, ib] layout the h-matmul needs)
    # pairs 8-15 run in the tail: the loop stays ACT-bound (612/tile) and
    # the deferred U_A matmuls fill the PE-idle h round-trip window
    maccT = spool.tile([P, NIB, P], BF16, tag="st2", name="maccT")
    for ib in range(NIB):
        nc.tensor.transpose(maccT[:, ib, :],
                            macc[:, ib * P:(ib + 1) * P], ident)
    # Z columns: tiny DoubleRow ones-matmuls straight into the [p, ib]
    # layout (all in the tail so no psum bank is held across the loop)
    z2 = spool.tile([P, NIB], F32, tag="st2", name="z2")
    for tp in range(NJT // 2):
        jt0 = 2 * tp
        for ib in range(NIB):
            nc.tensor.matmul(z2[:, ib:ib + 1],
                             lhsT=e8[:, jt0:jt0 + 2, ib * P:(ib + 1) * P],
                             rhs=ones8,
                             start=(tp == 0 and ib == 0),
                             stop=(tp == NJT // 2 - 1),
                             perf_mode=mybir.MatmulPerfMode.DoubleRow,
                             skip_group_check=True)
    emit_pair(12)
    emit_pair(13)

    maxe_pp = stat.tile([P, NIB], F32, tag="maxe", name="maxe_pp")
    nc.vector.tensor_reduce(out=maxe_pp, in_=maccT,
                            axis=mybir.AxisListType.X,
                            op=mybir.AluOpType.max)
    zinv = stat.tile([P, NIB], F32, tag="zinv", name="zinv")
    nc.vector.reciprocal(out=zinv, in_=z2)
    b_f = stat.tile([P, NIB], F32, tag="bf", name="b_f")
    nc.vector.tensor_mul(out=b_f, in0=maxe_pp, in1=zinv)
    b_bf = stat.tile([P, NIB], BF16, tag="bbf", name="b_bf")
    nc.vector.tensor_copy(out=b_bf, in_=b_f)

    # ---- h partial + AllReduce ------------------------------------------
    h_ps = spool.tile([P, NDC], F32, tag="st2", name="h_ps")
    for ib in range(NIB):
        for dc in range(NDC):
            nc.tensor.matmul(h_ps[:, dc:dc + 1],
                             lhsT=cb[:, ib, dc * P:(dc + 1) * P],
                             rhs=b_bf[:, ib:ib + 1],
                             start=(ib == 0 and dc == 0),
                             stop=(ib == NIB - 1 and dc == NDC - 1),
                             skip_group_check=True)
    h_sb = stat.tile([P, NDC], F32, tag="hsb", name="h_sb")
    nc.scalar.activation(out=h_sb, in_=h_ps, func=AF.Copy)
    for tp in range(14, NJT // 2):
        emit_pair(tp)
    hp_dram = dram.tile([D], F32)
    # h round-trip on the sync HWDGE queue: idle at this point, so its ring
    # processes all three hops back-to-back before the bulky G12 writes (on
    # the scalar queue) are even ready to compete for the DMA engines
    hp_ap = hp_dram[:]
    nc.sync.dma_start(out=hp_ap.rearrange("(dc p) -> p dc", p=P), in_=h_sb)
    if collective:
        # in-place AllReduce: the sim build then needs no stand-in copy hop
        nc.gpsimd.collective_compute(
            "AllReduce", mybir.AluOpType.add,
            replica_groups=[list(range(NCORES))],
            ins=[hp_dram.opt()], outs=[hp_dram.opt()],
        )
    h_bc = consts.tile([P, D], F32)
    nc.sync.dma_start(
        out=h_bc,
        in_=bass.AP(tensor=hp_ap.tensor, offset=hp_ap.offset,
                    ap=[[0, P], [1, D]]),
    )

    # ---- U_A^T -> U_A, G1..2 (independent of h; fills the AR window) -----
    uat = consts.tile([P, NDC, TL], BF16)   # [p(dq), dqc, i]
    for dqc in range(NDC):
        if dqc < 2:
            nc.scalar.activation(out=uat[:, dqc, :], in_=ua_ps[dqc],
                                 func=AF.Copy)
        else:
            nc.vector.tensor_copy(out=uat[:, dqc, :], in_=ua_ps[dqc])

    g12 = gout.tile([P, NIB, 2, D], BF16, tag="g12", name="g12")
    for ib in range(NIB):
        ps = spool.tile([P, D], BF16, tag="st2", name=f"uat{ib}")
        for dqc in range(NDC):
            nc.tensor.transpose(ps[:, dqc * P:(dqc + 1) * P],
                                uat[:, dqc, ib * P:(ib + 1) * P], ident)
        # G1 = U_A = U_A^T.T * zinv
        nc.scalar.activation(out=g12[:, ib, 0, :], in_=ps, func=AF.Copy,
                             scale=zinv[:, ib:ib + 1])
        # G2 = c * U_A
        nc.vector.tensor_mul(out=g12[:, ib, 1, :], in0=g12[:, ib, 0, :],
                             in1=cb[:, ib, :])
    # two half writes: [rows, 1024] G12 block, strided row AP puts the
    # 128-partition dim first so the cost counts only 2 ib-slices per DMA
    gfull = g123_dram[0:TL, 0:3 * D]
    for h in range(NIB):
        dst = bass.AP(tensor=gfull.tensor,
                      offset=gfull.offset + h * P * 3 * D,
                      ap=[[3 * D, P], [1, 2 * D]])
        nc.scalar.dma_start(out=dst, in_=g12[:, h, :, :])

    # ---- G3 = c * h (the only h-dependent work) --------------------------
    hbc16 = stat.tile([P, D], BF16, tag="hbc16", name="hbc16")
    nc.vector.tensor_copy(out=hbc16, in_=h_bc)
    g3 = gout.tile([P, NIB, D], BF16, tag="g3", name="g3")
    for hh in range(2):
        nc.vector.tensor_mul(
            out=g3[:, 2 * hh:2 * hh + 2, :], in0=cb[:, 2 * hh:2 * hh + 2, :],
            in1=bass.AP(tensor=hbc16.tensor, offset=hbc16.offset,
                        ap=[hbc16.ap[0], [0, 2], [1, D]]))
        dst3 = bass.AP(tensor=gfull.tensor,
                       offset=gfull.offset + 2 * D + hh * 2 * P * 3 * D,
                       ap=[[3 * D, P], [P * 3 * D, 2], [1, D]])
        nc.sync.dma_start(out=dst3, in_=g3[:, 2 * hh:2 * hh + 2, :])

    ctx.close()


_NC_CACHE = {}


def _get_nc():
    if "nc" not in _NC_CACHE:
        _NC_CACHE["nc"] = build_kernel()
    return _NC_CACHE["nc"]


def _prep_inputs(x, kernel):
    """Host-side layout prep shared by kernel() and test harnesses."""
    context = np.ascontiguousarray(x[0, 0]).astype(np.float32)   # (T, D)
    question = np.ascontiguousarray(x[1, 0]).astype(np.float32)  # (T, D)
    w = np.asarray(kernel, dtype=np.float32)
    w2 = w[D:2 * D]
    w3 = w[2 * D:3 * D]
    w2p = np.ascontiguousarray(w2.reshape(NDC, P).T)
    w3p = np.ascontiguousarray(w3.reshape(NDC, P).T)

    q8 = question.astype(ml_dtypes.float8_e4m3)
    qlo8 = (question - q8.astype(np.float32)).astype(ml_dtypes.float8_e4m3)
    # q8n[p, jt, dq] = q8[jt*128 + p, dq]
    q8n = np.ascontiguousarray(q8.reshape(NJT, P, D).transpose(1, 0, 2))
    # qth[p, dc, j] = q8[j, dc*128 + p]
    qth = np.ascontiguousarray(q8.T.reshape(NDC, P, T).transpose(1, 0, 2))
    qtl = np.ascontiguousarray(qlo8.T.reshape(NDC, P, T).transpose(1, 0, 2))

    shared = {"q8n": q8n, "qth": qth, "qtl": qtl, "w2p": w2p, "w3p": w3p}
    in_maps = []
    for core in range(NCORES):
        m = dict(shared)
        cl = np.ascontiguousarray(context[core * TL:(core + 1) * TL])
        m["c"] = cl
        m["cb"] = np.ascontiguousarray(
            cl.astype(ml_dtypes.bfloat16).reshape(NIB, P, D).transpose(1, 0, 2))
        in_maps.append(m)
    return in_maps


def _assemble(results):
    out = []
    for core in range(NCORES):
        g0 = np.asarray(results[core]["g0"], dtype=np.float32)
        g123 = np.asarray(results[core]["g123"]).astype(np.float32)
        out.append(np.concatenate([g0, g123], axis=1))
    return np.concatenate(out, axis=0)


def kernel(x: np.ndarray, kernel: np.ndarray) -> np.ndarray:
    nc = _get_nc()
    in_maps = _prep_inputs(x, kernel)
    res = run_bass_kernel_spmd(nc, in_maps, core_ids=list(range(NCORES)))
    return _assemble(res.results).astype(np.float32)


# revision 41
# speedup vs baseline: 1.0468x; 1.0468x over previous
"""BiAttention (BiDAF-style) kernel for Trainium2, 8 NeuronCores.

Reference math (T=4096, d=512):
    context  = x[0,0]; question = x[1,0]
    S[i,j]   = w1.c_i + w2.q_j + (c_i*w3).q_j
    A        = softmax_j(S)          # w1.c_i is constant per row -> cancels
    U_A      = A @ question
    b        = max_j A[i,j]
    h        = b @ context           # global over T -> one AllReduce
    G        = [context, U_A, context*U_A, context*h]

Sharding: context rows (rows of S/A/U_A/G) split across 8 cores (512 each);
question replicated; h all-reduced (2 KB).

Per-core compute strategy (all big GEMMs in fp8-e4m3 DoubleRow, 4x bf16
PE throughput; S computed TRANSPOSED so exp emits E^T directly and no
E transposes are needed):

  S^T[j,i] = sum_d q[j,d] * (c[i,d]*w3[d] + w2[d])
    - lhsT  = qT (d on partitions), host-pretransposed fp8 hi + lo parts
      (q = q8 + qlo8 error-compensation: halves the fp8 matmul noise,
      needed for the b/h accuracy budget)
    - rhs   = cw3T fp8 = (c^T * w3 + w2), from 16 PE transposes of bf16 c
    - 4 DoubleRow matmuls per 128-row j-tile (2 d-pairs x {hi,lo})
  E^T = exp(S^T - 2) -> bf16 SBUF (ACT; global shift keeps E in fp8 range,
    softmax/max ratios are shift-invariant)
  E8  = fp8(E^T)     (Pool/DVE copies; feeds the U_A GEMM)
  Z   = e8-chunk^T @ ones per i-block: tiny DoubleRow column matmuls give
        Z directly in the [p, ib] layout, no cross-partition reduce
  b   = max_j E^T / Z: DVE running tensor_max per tile, then 4 PE
        transposes of the [j-lane, i] partial max + one free-axis DVE max
        (bf16-accurate; an fp8-E max would blow the tolerance)
  U_A^T[dq,i] = sum_j q8[j,dq] * E8[j,i]  (lhsT = q natural fp8 - no
        transposes; 64 DoubleRow matmuls), then 16 PE transposes back
        and scale by 1/Z.
  h   = b @ c  (tiny bf16 matmuls, f32 psum, 2 KB in-place AllReduce)
  G0  = c exactly via DRAM->DRAM copy; G1..3 written bf16 (within the
        2e-2 budget) and upcast on host.

The schedule keeps every engine off the b->h->G3 critical path: the h
round-trip DMAs ride the otherwise-idle sync queue, output writes ride
the scalar queue, and the U_A pair matmuls trail their e8 casts by two
tiles so the in-order PE queue never head-of-line blocks.
"""

import numpy as np
import ml_dtypes

import concourse.bass as bass
import concourse.mybir as mybir
import concourse.tile as tile
from concourse import bacc
from concourse.bass_utils import run_bass_kernel_spmd
from concourse.masks import make_identity

F32 = mybir.dt.float32
BF16 = mybir.dt.bfloat16
F8 = mybir.dt.float8e4
AF = mybir.ActivationFunctionType

T = 4096
D = 512
NCORES = 8
TL = T // NCORES          # 512 local context rows per core
P = 128
NIB = TL // P             # 4 i-blocks of 128 rows
NJT = T // P              # 32 j-tiles of 128
NDC = D // P              # 4 d-chunks of 128
SHIFT = 2.0               # global logit shift: E = exp(S - 2) <= ~13


def build_kernel(collective=True, compile=True):
    nc = bacc.Bacc("TRN2", target_bir_lowering=False, debug=False,
                   num_devices=NCORES if collective else 1)

    c_dram = nc.dram_tensor("c", [TL, D], F32, kind="ExternalInput").ap()
    cb_dram = nc.dram_tensor("cb", [P, NIB, D], BF16, kind="ExternalInput").ap()
    q8n_dram = nc.dram_tensor("q8n", [P, NJT, D], F8, kind="ExternalInput").ap()
    qth_dram = nc.dram_tensor("qth", [P, NDC, T], F8, kind="ExternalInput").ap()
    qtl_dram = nc.dram_tensor("qtl", [P, NDC, T], F8, kind="ExternalInput").ap()
    w2p_dram = nc.dram_tensor("w2p", [P, NDC], F32, kind="ExternalInput").ap()
    w3p_dram = nc.dram_tensor("w3p", [P, NDC], F32, kind="ExternalInput").ap()
    g0_dram = nc.dram_tensor("g0", [TL, D], F32, kind="ExternalOutput").ap()
    g123_dram = nc.dram_tensor("g123", [TL, 3 * D], BF16,
                               kind="ExternalOutput").ap()

    with tile.TileContext(nc) as tc:
        _emit(nc, tc, c_dram, cb_dram, q8n_dram, qth_dram, qtl_dram, w2p_dram,
              w3p_dram, g0_dram, g123_dram, collective=collective)

    if compile:
        nc.compile()
    return nc


def _emit(nc, tc, c_dram, cb_dram, q8n_dram, qth_dram, qtl_dram, w2p_dram,
          w3p_dram, g0_dram, g123_dram, collective=True):
    from contextlib import ExitStack
    ctx = ExitStack()
    consts = ctx.enter_context(tc.tile_pool(name="consts", bufs=1))
    epool = ctx.enter_context(tc.tile_pool(name="epool", bufs=1))
    gout = ctx.enter_context(tc.tile_pool(name="gout", bufs=2))
    stat = ctx.enter_context(tc.tile_pool(name="stat", bufs=2))
    spool = ctx.enter_context(tc.tile_pool(name="spool", bufs=4, space="PSUM"))
    uapool = ctx.enter_context(tc.tile_pool(name="uapool", bufs=1, space="PSUM"))
    dram = ctx.enter_context(tc.tile_pool(name="dram", bufs=1, space="DRAM"))

    # ---- prologue --------------------------------------------------------
    # cb load: [p, ib, d] bf16, host-precast; gates cw3T which gates all S^T
    cb = consts.tile([P, NIB, D], BF16)
    nc.sync.dma_start(out=cb[:, :, 0:D // 2], in_=cb_dram[:, :, 0:D // 2])
    nc.sync.dma_start(out=cb[:, :, D // 2:D], in_=cb_dram[:, :, D // 2:D])

    ident = consts.tile([P, P], BF16)
    make_identity(nc, ident)

    # dummy exp: pull the ACT table load into the startup DMA window
    warm = consts.tile([1, 1], F32)
    nc.vector.memset(warm, 0.0)
    nc.scalar.activation(out=warm, in_=warm, func=AF.Exp)

    ebias = consts.tile([P, 1], F32)
    nc.vector.memset(ebias, -SHIFT)
    ones8 = consts.tile([P, 2, 1], F8)
    nc.vector.memset(ones8, 1.0)

    # HAM warm-up: ramp the PE clock while startup DMAs run
    wa = consts.tile([P, P], BF16)
    nc.vector.memset(wa, 0.0)
    wb = consts.tile([P, 512], BF16)
    nc.vector.memset(wb, 0.0)
    for wi in range(3):
        wps = uapool.tile([P, 512], F32, tag=f"ua{wi}", name=f"wps{wi}")
        nc.tensor.matmul(wps, lhsT=wa, rhs=wb, start=True, stop=True)

    # w first (tiny; gates the cw3T ACT drain), then 8 interleaved groups of
    # (qth, qtl) 4-tile slices with q8n trailing one group (UA lags anyway),
    # so the S^T pipeline starts after ~4 small DMAs instead of all of q
    w2p = consts.tile([P, NDC], F32)
    nc.sync.dma_start(out=w2p, in_=w2p_dram)
    w3p = consts.tile([P, NDC], F32)
    nc.sync.dma_start(out=w3p, in_=w3p_dram)
    qth = consts.tile([P, NDC, T], F8)
    qtl = consts.tile([P, NDC, T], F8)
    q8n = consts.tile([P, NJT, D], F8)
    nc.sync.dma_start(out=qth[:, :, 0:512], in_=qth_dram[:, :, 0:512])
    nc.sync.dma_start(out=qtl[:, :, 0:512], in_=qtl_dram[:, :, 0:512])
    for g in range(8):
        j0, j1 = g * 512, (g + 1) * 512
        if g >= 1:
            nc.sync.dma_start(out=qth[:, :, j0:j1], in_=qth_dram[:, :, j0:j1])
            nc.sync.dma_start(out=qtl[:, :, j0:j1], in_=qtl_dram[:, :, j0:j1])
        if g >= 1:
            nc.sync.dma_start(out=q8n[:, (g - 1) * 4:g * 4, :],
                              in_=q8n_dram[:, (g - 1) * 4:g * 4, :])
    nc.sync.dma_start(out=q8n[:, 28:32, :], in_=q8n_dram[:, 28:32, :])
    # G0 = context, exact f32, DRAM->DRAM; sync queue AFTER the q loads so
    # its (dependency-free) transfer cannot preempt the pipeline feed
    nc.sync.dma_start(out=g0_dram, in_=c_dram)

    # ---- cw3T8[d, i] = c^T * w3 + w2, fp8 --------------------------------
    cw3 = consts.tile([P, NDC, TL], F8)   # [p(d), dc, i]
    for dc in range(NDC):
        ps = uapool.tile([P, TL], BF16, tag=f"ua{dc}", name=f"tc{dc}")
        for ib in range(NIB):
            nc.tensor.transpose(ps[:, ib * P:(ib + 1) * P],
                                cb[:, ib, dc * P:(dc + 1) * P], ident)
        nc.scalar.activation(out=cw3[:, dc, :], in_=ps, func=AF.Identity,
                             bias=w2p[:, dc:dc + 1], scale=w3p[:, dc:dc + 1])

    # ---- persistent tiles ------------------------------------------------
    e16 = epool.tile([P, NJT, TL], BF16, tag="e16", name="e16")  # [j, jt, i]
    e8 = epool.tile([P, NJT, TL], F8, tag="e8", name="e8")
    macc = stat.tile([P, TL], BF16, tag="macc", name="macc")
    ua_ps = [uapool.tile([P, TL], F32, tag=f"ua{dqc}", name=f"ua{dqc}")
             for dqc in range(NDC)]

    # ---- main loop over j-tile pairs -------------------------------------
    def emit_pair(tp):
        """U_A DoubleRow matmuls for j-tile pair tp (tiles 2tp, 2tp+1)."""
        jt0 = 2 * tp
        for dqc in range(NDC):
            nc.tensor.matmul(
                ua_ps[dqc],
                lhsT=q8n[:, jt0:jt0 + 2, dqc * P:(dqc + 1) * P],
                rhs=e8[:, jt0:jt0 + 2, :],
                start=(tp == 0), stop=(tp == NJT // 2 - 1),
                perf_mode=mybir.MatmulPerfMode.DoubleRow,
                skip_group_check=True)

    for jt in range(NJT):
        st = spool.tile([P, TL], F32, tag="st2", name=f"st{jt}")
        m = 0
        for qt in (qth, qtl):
            for cp in range(2):
                nc.tensor.matmul(
                    st,
                    lhsT=qt[:, 2 * cp:2 * cp + 2, jt * P:(jt + 1) * P],
                    rhs=cw3[:, 2 * cp:2 * cp + 2, :],
                    start=(m == 0), stop=(m == 3),
                    perf_mode=mybir.MatmulPerfMode.DoubleRow)
                m += 1
        nc.scalar.activation(out=e16[:, jt, :], in_=st, func=AF.Exp,
                             bias=ebias, scale=1.0)
        if jt == 0:
            nc.vector.tensor_copy(out=macc, in_=e16[:, jt, :])
        else:
            nc.vector.tensor_max(out=macc, in0=macc, in1=e16[:, jt, :])
        cast_eng = nc.gpsimd if jt % 8 in (0, 1, 3, 4, 6) else nc.vector
        cast_eng.tensor_copy(out=e8[:, jt, :], in_=e16[:, jt, :])
        # UA pairs run behind the casts so the in-order PE queue never
        # head-of-line blocks on a fresh e8 cast; pairs 14/15 land after
        # the b/h launch to fill the AllReduce latency window with U_A work
        if jt % 2 == 1 and jt >= 5:
            emit_pair((jt - 5) // 2)

    # ---- stats: maxe via PE transpose of the running max (partition max
    # without gpsimd: transpose [j-lane, i] -> [i, j-lane], then a free-axis
    # DVE max directly in the [p, ib] layout the h-matmul needs)
    # pair 14 is ready at loop end - run it while DVE finishes the last
    # running-max that gates the maccT transposes
    emit_pair(NJT // 2 - 2)
    maccT = spool.tile([P, NIB, P], BF16, tag="st2", name="maccT")
    for ib in range(NIB):
        nc.tensor.transpose(maccT[:, ib, :],
                            macc[:, ib * P:(ib + 1) * P], ident)
    # Z columns: tiny DoubleRow ones-matmuls straight into the [p, ib]
    # layout (all in the tail so no psum bank is held across the loop)
    z2 = spool.tile([P, NIB], F32, tag="st2", name="z2")
    for tp in range(NJT // 2):
        jt0 = 2 * tp
        for ib in range(NIB):
            nc.tensor.matmul(z2[:, ib:ib + 1],
                             lhsT=e8[:, jt0:jt0 + 2, ib * P:(ib + 1) * P],
                             rhs=ones8,
                             start=(tp == 0 and ib == 0),
                             stop=(tp == NJT // 2 - 1),
                             perf_mode=mybir.MatmulPerfMode.DoubleRow,
                             skip_group_check=True)
    emit_pair(NJT // 2 - 1)

    maxe_pp = stat.tile([P, NIB], F32, tag="maxe", name="maxe_pp")
    nc.vector.tensor_reduce(out=maxe_pp, in_=maccT,
                            axis=mybir.AxisListType.X,
                            op=mybir.AluOpType.max)
    zinv = stat.tile([P, NIB], F32, tag="zinv", name="zinv")
    nc.vector.reciprocal(out=zinv, in_=z2)
    b_f = stat.tile([P, NIB], F32, tag="bf", name="b_f")
    nc.vector.tensor_mul(out=b_f, in0=maxe_pp, in1=zinv)
    b_bf = stat.tile([P, NIB], BF16, tag="bbf", name="b_bf")
    nc.vector.tensor_copy(out=b_bf, in_=b_f)

    # ---- h partial + AllReduce ------------------------------------------
    h_ps = spool.tile([P, NDC], F32, tag="st2", name="h_ps")
    for ib in range(NIB):
        for dc in range(NDC):
            nc.tensor.matmul(h_ps[:, dc:dc + 1],
                             lhsT=cb[:, ib, dc * P:(dc + 1) * P],
                             rhs=b_bf[:, ib:ib + 1],
                             start=(ib == 0 and dc == 0),
                             stop=(ib == NIB - 1 and dc == NDC - 1),
                             skip_group_check=True)
    h_sb = stat.tile([P, NDC], F32, tag="hsb", name="h_sb")
    nc.scalar.activation(out=h_sb, in_=h_ps, func=AF.Copy)
    hp_dram = dram.tile([D], F32)
    # h round-trip on the sync HWDGE queue: idle at this point, so its ring
    # processes all three hops back-to-back before the bulky G12 writes (on
    # the scalar queue) are even ready to compete for the DMA engines
    hp_ap = hp_dram[:]
    nc.sync.dma_start(out=hp_ap.rearrange("(dc p) -> p dc", p=P), in_=h_sb)
    if collective:
        # in-place AllReduce: the sim build then needs no stand-in copy hop
        nc.gpsimd.collective_compute(
            "AllReduce", mybir.AluOpType.add,
            replica_groups=[list(range(NCORES))],
            ins=[hp_dram.opt()], outs=[hp_dram.opt()],
        )
    h_bc = consts.tile([P, D], F32)
    nc.sync.dma_start(
        out=h_bc,
        in_=bass.AP(tensor=hp_ap.tensor, offset=hp_ap.offset,
                    ap=[[0, P], [1, D]]),
    )

    # ---- U_A^T -> U_A, G1..2 (independent of h; fills the AR window) -----
    uat = consts.tile([P, NDC, TL], BF16)   # [p(dq), dqc, i]
    for dqc in range(NDC):
        nc.scalar.activation(out=uat[:, dqc, :], in_=ua_ps[dqc], func=AF.Copy)

    g12 = gout.tile([P, NIB, 2, D], BF16, tag="g12", name="g12")
    for ib in range(NIB):
        ps = spool.tile([P, D], BF16, tag="st2", name=f"uat{ib}")
        for dqc in range(NDC):
            nc.tensor.transpose(ps[:, dqc * P:(dqc + 1) * P],
                                uat[:, dqc, ib * P:(ib + 1) * P], ident)
        # G1 = U_A = U_A^T.T * zinv
        nc.scalar.activation(out=g12[:, ib, 0, :], in_=ps, func=AF.Copy,
                             scale=zinv[:, ib:ib + 1])
        # G2 = c * U_A
        nc.vector.tensor_mul(out=g12[:, ib, 1, :], in0=g12[:, ib, 0, :],
                             in1=cb[:, ib, :])
    # two half writes: [rows, 1024] G12 block, strided row AP puts the
    # 128-partition dim first so the cost counts only 2 ib-slices per DMA
    gfull = g123_dram[0:TL, 0:3 * D]
    for h in range(NIB):
        dst = bass.AP(tensor=gfull.tensor,
                      offset=gfull.offset + h * P * 3 * D,
                      ap=[[3 * D, P], [1, 2 * D]])
        nc.scalar.dma_start(out=dst, in_=g12[:, h, :, :])

    # ---- G3 = c * h (the only h-dependent work) --------------------------
    hbc16 = stat.tile([P, D], BF16, tag="hbc16", name="hbc16")
    nc.vector.tensor_copy(out=hbc16, in_=h_bc)
    g3 = gout.tile([P, NIB, D], BF16, tag="g3", name="g3")
    for hh in range(2):
        nc.vector.tensor_mul(
            out=g3[:, 2 * hh:2 * hh + 2, :], in0=cb[:, 2 * hh:2 * hh + 2, :],
            in1=bass.AP(tensor=hbc16.tensor, offset=hbc16.offset,
                        ap=[hbc16.ap[0], [0, 2], [1, D]]))
        dst3 = bass.AP(tensor=gfull.tensor,
                       offset=gfull.offset + 2 * D + hh * 2 * P * 3 * D,
                       ap=[[3 * D, P], [P * 3 * D, 2], [1, D]])
        nc.sync.dma_start(out=dst3, in_=g3[:, 2 * hh:2 * hh + 2, :])

    ctx.close()


_NC_CACHE = {}


def _get_nc():
    if "nc" not in _NC_CACHE:
        _NC_CACHE["nc"] = build_kernel()
    return _NC_CACHE["nc"]


def _prep_inputs(x, kernel):
    """Host-side layout prep shared by kernel() and test harnesses."""
    context = np.ascontiguousarray(x[0, 0]).astype(np.float32)   # (T, D)
    question = np.ascontiguousarray(x[1, 0]).astype(np.float32)  # (T, D)
    w = np.asarray(kernel, dtype=np.float32)
    w2 = w[D:2 * D]
    w3 = w[2 * D:3 * D]
    w2p = np.ascontiguousarray(w2.reshape(NDC, P).T)
    w3p = np.ascontiguousarray(w3.reshape(NDC, P).T)

    q8 = question.astype(ml_dtypes.float8_e4m3)
    qlo8 = (question - q8.astype(np.float32)).astype(ml_dtypes.float8_e4m3)
    # q8n[p, jt, dq] = q8[jt*128 + p, dq]
    q8n = np.ascontiguousarray(q8.reshape(NJT, P, D).transpose(1, 0, 2))
    # qth[p, dc, j] = q8[j, dc*128 + p]
    qth = np.ascontiguousarray(q8.T.reshape(NDC, P, T).transpose(1, 0, 2))
    qtl = np.ascontiguousarray(qlo8.T.reshape(NDC, P, T).transpose(1, 0, 2))

    shared = {"q8n": q8n, "qth": qth, "qtl": qtl, "w2p": w2p, "w3p": w3p}
    in_maps = []
    for core in range(NCORES):
        m = dict(shared)
        cl = np.ascontiguousarray(context[core * TL:(core + 1) * TL])
        m["c"] = cl
        m["cb"] = np.ascontiguousarray(
            cl.astype(ml_dtypes.bfloat16).reshape(NIB, P, D).transpose(1, 0, 2))
        in_maps.append(m)
    return in_maps


def _assemble(results):
    out = []
    for core in range(NCORES):
        g0 = np.asarray(results[core]["g0"], dtype=np.float32)
        g123 = np.asarray(results[core]["g123"]).astype(np.float32)
        out.append(np.concatenate([g0, g123], axis=1))
    return np.concatenate(out, axis=0)


def kernel(x: np.ndarray, kernel: np.ndarray) -> np.ndarray:
    nc = _get_nc()
    in_maps = _prep_inputs(x, kernel)
    res = run_bass_kernel_spmd(nc, in_maps, core_ids=list(range(NCORES)))
    return _assemble(res.results).astype(np.float32)
